# revision 1
# baseline (speedup 1.0000x reference)
"""Trainium2 Bass kernel for nn_GAT_37580963840365 (2-layer TransformerConv GNN + MLP).

Strategy (8 NeuronCores, dst-sharded):
 - Nodes split 12500/core. Per core, nodes are bin-packed into NBLK blocks of
   128 such that each (block, src-quadrant) bucket has <=256 edges. Edges are
   permuted into 128-edge tiles that are (block, quadrant)-pure; the tile
   structure is shared (max) across cores so one SPMD program serves all 8.
 - Per tile, attention runs as: one-hot matmuls (Sel = expand per-dst q to
   per-edge, O = scatter per-edge messages to the block's 128 nodes), DVE
   elementwise for alpha/exp, PSUM accumulation of [exv | ex | ex*ea].
 - Softmax max-subtraction is dropped (validated: exp stays in f32/bf16
   range); per-dst additive alpha terms (q . bk) cancel in softmax and are
   dropped; rank-1 edge-feature term (ea * We) is folded analytically.
 - Layer 0 exploits the 2-dim input: per-edge messages are rank-3 in x, so no
   k/v gathers at all. Layer 1 gathers fp16 [k|v] rows with dma_gather.
 - Two launches (layer0, then layer1+classifier) with a host-side h1
   all-gather between them (cheaper than on-device collectives).
"""

import sys, os
for _p in ("/opt/trn_rl_repo", "/root/.axon_site/_ro/trn_rl_repo"):
    if os.path.isdir(_p) and _p not in sys.path:
        sys.path.append(_p)

import numpy as np
import ml_dtypes
from contextlib import ExitStack

import concourse.bass as bass
import concourse.bacc as bacc
import concourse.tile as tile
from concourse import mybir
from concourse.bass_utils import run_bass_kernel_spmd

f32 = mybir.dt.float32
f16 = mybir.dt.float16
bf16 = mybir.dt.bfloat16
i16 = mybir.dt.int16
AF = mybir.ActivationFunctionType
AX = mybir.AxisListType

H, C = 4, 32
HC = H * C
SQC = float(np.sqrt(C))


# ----------------------------------------------------------------- host prep

class Cfg:
    def __init__(self, N, E, ncores, nblk, gblk, qs):
        self.N, self.E, self.NCORES = N, E, ncores
        self.NBLK, self.GBLK, self.QS = nblk, gblk, qs
        self.NQ = 4
        self.NPC = N // ncores               # real nodes per core
        self.NLOC = nblk * 128               # padded local nodes (packed order)
        self.NGLOB = ((N + 127) // 128) * 128  # padded global rows for kv table


FULL = Cfg(N=100000, E=800000, ncores=8, nblk=100, gblk=2, qs=25000)


def _pack_blocks(deg, nblk):
    """Greedy vector bin-packing: nodes (rows of deg [n,4]) into nblk blocks of
    <=128 nodes, balancing per-quadrant edge loads. Returns blockof [n]."""
    n = deg.shape[0]
    order = np.argsort(-deg.sum(1), kind="stable")
    loads = np.zeros((nblk, 4), np.int64)
    counts = np.zeros(nblk, np.int64)
    blockof = np.empty(n, np.int64)
    # process heavy nodes individually, light nodes in bulk round-robin
    heavy = order[: min(n, 4000)]
    light = order[min(n, 4000):]
    CAP = 256
    for nd in heavy:
        new = loads + deg[nd]
        feas = (counts < 128) & (new <= CAP).all(1)
        score = new.max(1)
        if feas.any():
            score = np.where(feas, score, 1 << 30)
        else:
            score = np.where(counts < 128, (new - CAP).clip(0).sum(1) * 1000 + score,
                             1 << 30)
        j = int(np.argmin(score))
        blockof[nd] = j
        loads[j] += deg[nd]
        counts[j] += 1
    for nd in light:
        new = loads + deg[nd]
        feas = (counts < 128) & (new <= CAP).all(1)
        score = new.max(1) + counts * 0.02
        if feas.any():
            score = np.where(feas, score, 1 << 30)
        else:
            score = np.where(counts < 128, (new - CAP).clip(0).sum(1) * 1000 + score,
                             1 << 30)
        j = int(np.argmin(score))
        blockof[nd] = j
        loads[j] += deg[nd]
        counts[j] += 1
    return blockof, loads, counts


class Plan:
    pass


def make_plan(cfg, src, dst):
    """Build shared tile structure + per-core packed edge arrays."""
    NC, NPC, NBLK, NQ, QS = cfg.NCORES, cfg.NPC, cfg.NBLK, cfg.NQ, cfg.QS
    cores = []
    all_loads = np.zeros((NC, NBLK, NQ), np.int64)
    for i in range(NC):
        m = (dst >= i * NPC) & (dst < (i + 1) * NPC)
        s, d = src[m], dst[m] - i * NPC
        q = s // QS
        deg = np.zeros((NPC, NQ), np.int64)
        np.add.at(deg, (d, q), 1)
        blockof, loads, counts = _pack_blocks(deg, NBLK)
        # canonical block order: by total load desc (aligns shapes across cores)
        bord = np.argsort(-loads.sum(1), kind="stable")
        inv = np.empty(NBLK, np.int64)
        inv[bord] = np.arange(NBLK)
        blockof = inv[blockof]
        loads = loads[bord]
        # node list per block, padded with -1
        blocknodes = np.full((NBLK, 128), -1, np.int64)
        nodepos = np.empty(NPC, np.int64)
        for b in range(NBLK):
            nds = np.where(blockof == b)[0]
            blocknodes[b, : len(nds)] = nds
            nodepos[nds] = np.arange(len(nds))
        cores.append(dict(s=s, d=d, q=q, blockof=blockof, nodepos=nodepos,
                          blocknodes=blocknodes, gmask=np.where(m)[0]))
        all_loads[i] = loads

    tpq = np.ceil(all_loads / 128.0).astype(np.int64).max(0)  # [NBLK, NQ] shared

    # shared slot layout: groups of GBLK blocks; order (g, q, b, k)
    tiles = []          # list of dicts: b, q, g, pos (tile idx within group)
    gathers = []        # per (g, q): dict(g,q,tile0,ntiles)
    groups = []         # per g: dict(blocks, tile0, ntiles, gathers)
    nblk, gblk = NBLK, cfg.GBLK
    t_global = 0
    for g0 in range(0, nblk, gblk):
        blocks = list(range(g0, min(g0 + gblk, nblk)))
        ginfo = dict(blocks=blocks, tile0=t_global, kvg=[], g=len(groups))
        for q in range(NQ):
            gt0 = t_global
            for b in blocks:
                for k in range(int(tpq[b, q])):
                    tiles.append(dict(b=b, q=q, g=ginfo["g"],
                                      pos=t_global - ginfo["tile0"]))
                    t_global += 1
            if t_global > gt0:
                ginfo["kvg"].append(dict(q=q, tile0=gt0, ntiles=t_global - gt0))
        ginfo["ntiles"] = t_global - ginfo["tile0"]
        assert ginfo["ntiles"] > 0
        groups.append(ginfo)
    NT = t_global

    # per-block first/last tile (for psum start/stop) in global tile order
    first_tile = {}
    last_tile = {}
    for t, ti in enumerate(tiles):
        key = ti["b"]
        if key not in first_tile:
            first_tile[key] = t
        last_tile[key] = t

    # slot base per (b, q) bucket
    bucket_base = {}
    cursor = {}
    for t, ti in enumerate(tiles):
        key = (ti["b"], ti["q"])
        if key not in bucket_base:
            bucket_base[key] = t * 128
    # per-core arrays
    pc = []
    for i in range(NC):
        cd = cores[i]
        srcidx = np.zeros(NT * 128, np.int16)
        dstloc = np.full(NT * 128, 128, np.int16)
        eav = np.zeros(NT * 128, np.float32)
        xs = np.zeros((NT * 128, 2), np.float32)
        okey = cd["blockof"][cd["d"]] * NQ + cd["q"]
        eorder = np.argsort(okey, kind="stable")
        ordered_keys, cnts = np.unique(okey[eorder], return_counts=True)
        off = 0
        positions = np.empty(len(eorder), np.int64)
        for key, cnt in zip(ordered_keys, cnts):
            b, q = int(key) // NQ, int(key) % NQ
            base = bucket_base[(b, q)]
            positions[off:off + cnt] = base + np.arange(cnt)
            off += cnt
        e_ids = eorder
        srcidx[positions] = (cd["s"][e_ids] - cd["q"][e_ids] * QS).astype(np.int16)
        dstloc[positions] = cd["nodepos"][cd["d"][e_ids]].astype(np.int16)
        qidx = np.full(NT * 128, cfg.NLOC, np.int16)  # padding -> zeros row
        qidx[positions] = (cd["blockof"][cd["d"][e_ids]] * 128 +
                           cd["nodepos"][cd["d"][e_ids]]).astype(np.int16)
        pc.append(dict(positions=positions, e_ids=e_ids, srcidx=srcidx,
                       dstloc=dstloc, qidx=qidx, core=cd))

    plan = Plan()
    plan.cfg = cfg
    plan.tiles, plan.groups, plan.NT = tiles, groups, NT
    plan.first_tile, plan.last_tile = first_tile, last_tile
    plan.cores = pc
    return plan


def wrap16(a):
    """int16 [n] -> [128, n//16] gather-idx layout (p = i%16, col = i//16,
    replicated across the 8 Q7 partition groups)."""
    return np.tile(a.reshape(-1, 16).T, (8, 1)).copy()


def head_block(v):
    """[HC] vector -> [H*? , HC] head-masked rows stacked: returns [H, HC] with
    row h = v masked to head h."""
    out = np.zeros((H, HC), np.float32)
    for h in range(H):
        out[h, h * C:(h + 1) * C] = v[h * C:(h + 1) * C]
    return out


# ------------------------------------------------------------ program builders

def _idx_tiles(tc, pool, plan, dst_only):
    """DMA resident idx/edge arrays. Returns dict of sbuf APs."""
    nc = tc.nc
    NT = plan.NT
    r = {}
    dstw = pool.tile([128, NT * 8], i16, name="dstw_sb")
    nc.sync.dma_start(dstw[:], tc._dram["dstw"][:, :])
    r["dstw"] = dstw
    if not dst_only:
        srcw = pool.tile([128, NT * 8], i16, name="srcw_sb")
        nc.sync.dma_start(srcw[:], tc._dram["srcw"][:, :])
        r["srcw"] = srcw
        ea = pool.tile([128, NT], f16, name="ea_sb")
        nc.sync.dma_start(ea[:], tc._dram["ea16"][:, :])
        r["ea"] = ea
    else:
        u4 = pool.tile([128, NT, 4], f16, name="u4_sb")
        nc.sync.dma_start(u4[:], tc._dram["u4"][:, :, :])
        r["u4"] = u4
    return r


def build_l0(nc, plan, stop=5):
    cfg = plan.cfg
    NT, NLOC, NBLK = plan.NT, cfg.NLOC, cfg.NBLK
    dram = {}
    def din(name, shape, dt):
        dram[name] = nc.dram_tensor(name, shape, dt, kind="ExternalInput").ap()
        return dram[name]
    xT1 = din("xT1", [4, NLOC], f16)
    din("dstw", [128, NT * 8], i16)
    din("u4", [128, NT, 4], f16)
    I_t = din("I_t", [256, 128], f16)
    Ib_t = din("Ib_t", [256, 128], bf16)
    A0 = din("A0", [4, 12], f16)
    V4 = din("V4", [16, 128], f16)
    Wsk0 = din("Wsk0", [4, 128], f16)
    din("qidxw", [128, NT * 8], i16)
    q4_t = nc.dram_tensor("q4_t", [NLOC + 128, 128], f16).ap()
    h1out = nc.dram_tensor("h1out", [NLOC, 128], f16, kind="ExternalOutput").ap()

    with tile.TileContext(nc) as tc, ExitStack() as ctx:
        tc._dram = dram
        res = ctx.enter_context(tc.tile_pool(name="res", bufs=1))
        gpool = ctx.enter_context(tc.tile_pool(name="gath", bufs=2))
        spool = ctx.enter_context(tc.tile_pool(name="small", bufs=3))
        qcpool = ctx.enter_context(tc.tile_pool(name="qcat", bufs=4))
        bpool = ctx.enter_context(tc.tile_pool(name="batch", bufs=3))
        fpool = ctx.enter_context(tc.tile_pool(name="fin", bufs=3))
        pq = ctx.enter_context(tc.tile_pool(name="pq", bufs=2, space="PSUM"))
        pagg = ctx.enter_context(tc.tile_pool(name="pagg", bufs=3, space="PSUM"))
        pfin = ctx.enter_context(tc.tile_pool(name="pfin", bufs=2, space="PSUM"))
        ptp = ctx.enter_context(tc.tile_pool(name="ptp", bufs=1, space="PSUM"))

        arrs = _idx_tiles(tc, res, plan, dst_only=True)
        dstw, u4 = arrs["dstw"], arrs["u4"]
        qiw = res.tile([128, NT * 8], i16, name="qiw_sb")
        nc.sync.dma_start(qiw[:], dram["qidxw"][:, :])
        xT1s = res.tile([4, NLOC], f16, name="xT1_sb")
        nc.sync.dma_start(xT1s[:], xT1[:, :])
        A0s = res.tile([4, 12], f16, name="A0_sb")
        nc.sync.dma_start(A0s[:], A0[:, :])
        V4s = res.tile([16, 128], f16, name="V4_sb")
        nc.sync.dma_start(V4s[:], V4[:, :])
        Wsk0s = res.tile([4, 128], f16, name="Wsk0_sb")
        nc.sync.dma_start(Wsk0s[:], Wsk0[:, :])
        ident = res.tile([128, 128], f16, name="ident_sb")
        nc.sync.dma_start(ident[:], I_t[0:128, :])

        # build q4 table (per-dst q combos) in DRAM
        for b in range(NBLK + 1):
            stg = qcpool.tile([128, 128], f16, tag="q4stg")
            nc.gpsimd.memset(stg[:], 0.0)
            if b < NBLK:
                ps = pq.tile([128, 12], f32, tag="pq")
                nc.tensor.matmul(ps[:], xT1s[:, b * 128:(b + 1) * 128], A0s[:],
                                 start=True, stop=True)
                nc.vector.tensor_copy(stg[:, 0:12], ps[:])
            nc.sync.dma_start(q4_t[b * 128:(b + 1) * 128, :], stg[:])
        for gi in plan.groups:
            g, blocks, tile0, Tg = gi["g"], gi["blocks"], gi["tile0"], gi["ntiles"]
            if stop < 2:
                break
            Ob = gpool.tile([128, Tg, 128], bf16, tag="Ob")
            qd = gpool.tile([128, Tg, 128], f16, tag="qd")
            for c0 in range(0, Tg, 8):
                cn = min(8, Tg - c0)
                nc.gpsimd.dma_gather(Ob[:, c0:c0 + cn, :], Ib_t[:, :],
                                     dstw[:, (tile0 + c0) * 8:(tile0 + c0 + cn) * 8],
                                     cn * 128, cn * 128, 128)
                nc.gpsimd.dma_gather(qd[:, c0:c0 + cn, :], q4_t[:, :],
                                     qiw[:, (tile0 + c0) * 8:(tile0 + c0 + cn) * 8],
                                     cn * 128, cn * 128, 128)
            if stop < 3:
                continue
            aggs = {}
            for b in blocks:
                aggs[b] = pagg.tile([128, 16], f32, tag="agg", name=f"agg{b}")
            # tiles, NB=4 batches
            t = 0
            if stop < 4:
                continue
            while t < Tg:
                nb = min(4, Tg - t)
                qj = bpool.tile([128, 4, 4, 3], f32, tag="qj")
                exb = bpool.tile([128, 4, 4], f32, tag="exb")
                rhs = bpool.tile([128, 4, 16], bf16, tag="rhs")
                nc.vector.tensor_mul(
                    qj[:, 0:nb, :, :],
                    qd[:, t:t + nb, 0:12].rearrange("p t (h j) -> p t h j", j=3),
                    u4[:, tile0 + t:tile0 + t + nb, 0:3].unsqueeze(2).broadcast_to([128, nb, 4, 3]))
                nc.vector.reduce_sum(exb[:, 0:nb, :], qj[:, 0:nb, :, :], axis=AX.X)
                nc.scalar.activation(exb[:, 0:nb, :], exb[:, 0:nb, :], AF.Exp)
                nc.vector.tensor_mul(
                    rhs[:, 0:nb, :].rearrange("p t (j h) -> p t j h", j=4),
                    exb[:, 0:nb, :].unsqueeze(2).broadcast_to([128, nb, 4, 4]),
                    u4[:, tile0 + t:tile0 + t + nb, :].unsqueeze(3).broadcast_to([128, nb, 4, 4]))
                for j in range(nb):
                    ti = plan.tiles[tile0 + t + j]
                    tg = tile0 + t + j
                    nc.tensor.matmul(aggs[ti["b"]][:], Ob[:, t + j, :], rhs[:, j, :],
                                     start=(tg == plan.first_tile[ti["b"]]),
                                     stop=(tg == plan.last_tile[ti["b"]]))
                t += nb
            # finalize blocks
            if stop < 5:
                continue
            for b in blocks:
                A = aggs[b]
                den = spool.tile([128, 4], f32, tag="den")
                nc.vector.tensor_scalar_add(den[:], A[:, 12:16], 1e-16)
                rec = spool.tile([128, 4], f32, tag="rec")
                nc.vector.reciprocal(rec[:], den[:])
                a4n = fpool.tile([128, 16], f16, tag="a4n")
                nc.vector.tensor_mul(
                    a4n[:].rearrange("p (j h) -> p j h", j=4),
                    A[:, 0:16].rearrange("p (j h) -> p j h", j=4),
                    rec[:].unsqueeze(1).broadcast_to([128, 4, 4]))
                tpa = ptp.tile([16, 128], f16, tag="tpa")
                nc.tensor.transpose(tpa[:], a4n[:], ident[:])
                a4nT = fpool.tile([16, 128], f16, tag="a4nT")
                nc.scalar.copy(a4nT[:], tpa[:])
                ps2 = pfin.tile([128, 128], f32, tag="pfin")
                nc.tensor.matmul(ps2[:], a4nT[:], V4s[:], start=True, stop=False)
                nc.tensor.matmul(ps2[:], xT1s[:, b * 128:(b + 1) * 128], Wsk0s[:],
                                 start=False, stop=True)
                h1b = fpool.tile([128, 128], f16, tag="h1b")
                nc.scalar.activation(h1b[:], ps2[:], AF.Relu)
                nc.sync.dma_start(h1out[b * 128:(b + 1) * 128, :], h1b[:])
    nc.compile()
    return nc


def build_l1(nc, plan, bc3: float):
    cfg = plan.cfg
    NT, NLOC, NBLK, NG, QS = plan.NT, cfg.NLOC, cfg.NBLK, cfg.NGLOB, cfg.QS
    dram = {}
    def din(name, shape, dt):
        dram[name] = nc.dram_tensor(name, shape, dt, kind="ExternalInput").ap()
        return dram[name]
    h1Tg = din("h1Tg", [128, NG], f16)
    h1Tl = din("h1Tl", [128, NLOC], f16)
    din("srcw", [128, NT * 8], i16)
    din("dstw", [128, NT * 8], i16)
    din("ea16", [128, NT], f16)
    I_t = din("I_t", [256, 128], f16)
    Ib_t = din("Ib_t", [256, 128], bf16)
    Wkv = din("Wkv", [128, 256], f16)
    Wqc = din("Wqc", [128, 132], f16)
    qbias = din("qbias", [1, 132], f16)
    Ws1 = din("Ws1", [128, 128], f16)
    brow1 = din("brow1", [1, 128], f16)
    We1bd = din("We1bd", [4, 128], f16)
    Wc1 = din("Wc1i", [128, 128], f16)
    Wc2 = din("Wc2i", [128, 64], f16)
    Wc3 = din("Wc3i", [64, 128], f16)   # only col 0 used (padded for DMA ease)
    bc1 = din("bc1i", [128, 1], f32)
    bc2 = din("bc2i", [64, 1], f32)
    din("qidxw", [128, NT * 8], i16)
    kv_t = nc.dram_tensor("kv_t", [NG, 256], f16).ap()
    qc_t = nc.dram_tensor("qc_t", [NLOC + 128, 256], f16).ap()
    outv = nc.dram_tensor("outv", [1, NLOC], f32, kind="ExternalOutput").ap()

    with tile.TileContext(nc) as tc, ExitStack() as ctx:
        tc._dram = dram
        res = ctx.enter_context(tc.tile_pool(name="res", bufs=1))
        arrs = _idx_tiles(tc, res, plan, dst_only=False)
        dstw, srcw, eas = arrs["dstw"], arrs["srcw"], arrs["ea"]
        h1Tls = res.tile([128, NLOC], f16, name="h1Tl_sb")
        nc.sync.dma_start(h1Tls[:], h1Tl[:, :])
        qiw = res.tile([128, NT * 8], i16, name="qiw_sb")
        nc.sync.dma_start(qiw[:], dram["qidxw"][:, :])
        h2T = res.tile([128, NLOC], f16, name="h2T_sb")
        outrow = res.tile([1, NLOC], f32, name="outrow_sb")
        Wkvs = res.tile([128, 256], f16, name="Wkv_sb")
        nc.sync.dma_start(Wkvs[:], Wkv[:, :])
        Wqcs = res.tile([128, 132], f16, name="Wqc_sb")
        nc.sync.dma_start(Wqcs[:], Wqc[:, :])
        qbias_s = res.tile([1, 132], f16, name="qbias_sb")
        nc.sync.dma_start(qbias_s[:], qbias[:, :])
        Ws1s = res.tile([128, 128], f16, name="Ws1_sb")
        nc.sync.dma_start(Ws1s[:], Ws1[:, :])
        brow1s = res.tile([1, 128], f16, name="brow1_sb")
        nc.sync.dma_start(brow1s[:], brow1[:, :])
        We1bds = res.tile([4, 128], f16, name="We1bd_sb")
        nc.sync.dma_start(We1bds[:], We1bd[:, :])
        ident = res.tile([128, 128], f16, name="ident_sb")
        nc.sync.dma_start(ident[:], I_t[0:128, :])
        ones1 = res.tile([1, 128], f16, name="ones1_sb")
        nc.gpsimd.memset(ones1[:], 1.0)

        # ---- phase 1: kv table build (all N rows, replicated work per core)
        with ExitStack() as c2:
            hp = c2.enter_context(tc.tile_pool(name="hchunk", bufs=3))
            kp = c2.enter_context(tc.tile_pool(name="kvrow", bufs=3))
            pkv = c2.enter_context(tc.tile_pool(name="pkv", bufs=4, space="PSUM"))
            for ch in range(NG // 128):
                hs = hp.tile([128, 128], f16, tag="h")
                nc.sync.dma_start(hs[:], h1Tg[:, ch * 128:(ch + 1) * 128])
                ps = pkv.tile([128, 256], f32, tag="pkv")
                nc.tensor.matmul(ps[:], hs[:], Wkvs[:], start=True, stop=True)
                ks = kp.tile([128, 256], f16, tag="kv")
                if ch % 2 == 0:
                    nc.scalar.copy(ks[:], ps[:])
                else:
                    nc.vector.tensor_copy(ks[:], ps[:])
                nc.sync.dma_start(kv_t[ch * 128:(ch + 1) * 128, :], ks[:])
            # local per-dst q table [q(128) | qWe(4) | pad]
            NBLK_ = NLOC // 128
            for b in range(NBLK_ + 1):
                stg = kp.tile([128, 256], f16, tag="qstg")
                nc.gpsimd.memset(stg[:], 0.0)
                if b < NBLK_:
                    ps = pkv.tile([128, 132], f32, tag="pqc")
                    nc.tensor.matmul(ps[:], h1Tls[:, b * 128:(b + 1) * 128], Wqcs[:],
                                     start=True, stop=False)
                    nc.tensor.matmul(ps[:], ones1[:], qbias_s[:], start=False, stop=True)
                    nc.vector.tensor_copy(stg[:, 0:132], ps[:])
                nc.sync.dma_start(qc_t[b * 128:(b + 1) * 128, :], stg[:])

        # ---- phase 2: attention
        with ExitStack() as c2:
            gpool = c2.enter_context(tc.tile_pool(name="gath", bufs=2))
            bpool = c2.enter_context(tc.tile_pool(name="batch", bufs=3))
            spool = c2.enter_context(tc.tile_pool(name="small", bufs=3))
            fpool = c2.enter_context(tc.tile_pool(name="fin", bufs=3))
            pagg = c2.enter_context(tc.tile_pool(name="pagg", bufs=3, space="PSUM"))
            pfin = c2.enter_context(tc.tile_pool(name="pfin", bufs=1, space="PSUM"))
            ptp = c2.enter_context(tc.tile_pool(name="ptp", bufs=1, space="PSUM"))

            for gi in plan.groups:
                blocks, tile0, Tg = gi["blocks"], gi["tile0"], gi["ntiles"]
                kvb = gpool.tile([128, Tg, 256], f16, tag="kvb")
                for kg in gi["kvg"]:
                    q, kt0, knt = kg["q"], kg["tile0"], kg["ntiles"]
                    for c0 in range(0, knt, 8):
                        cn = min(8, knt - c0)
                        nc.gpsimd.dma_gather(
                            kvb[:, kt0 - tile0 + c0:kt0 - tile0 + c0 + cn, :],
                            kv_t[q * QS:NG, :],
                            srcw[:, (kt0 + c0) * 8:(kt0 + c0 + cn) * 8],
                            cn * 128, cn * 128, 256)
                Ob = gpool.tile([128, Tg, 128], bf16, tag="Ob")
                qdg = gpool.tile([128, Tg, 256], f16, tag="qdg")
                for c0 in range(0, Tg, 8):
                    cn = min(8, Tg - c0)
                    nc.gpsimd.dma_gather(Ob[:, c0:c0 + cn, :], Ib_t[:, :],
                                         dstw[:, (tile0 + c0) * 8:(tile0 + c0 + cn) * 8],
                                         cn * 128, cn * 128, 128)
                    nc.gpsimd.dma_gather(qdg[:, c0:c0 + cn, :], qc_t[:, :],
                                         qiw[:, (tile0 + c0) * 8:(tile0 + c0 + cn) * 8],
                                         cn * 128, cn * 128, 256)
                aggs = {}
                for b in blocks:
                    aggs[b] = pagg.tile([128, 136], f32, tag="agg", name=f"agg{b}")
                t = 0
                while t < Tg:
                    nb = min(4, Tg - t)
                    qe = bpool.tile([128, 4, 8], f32, tag="qe")
                    ee = bpool.tile([128, 4, 8], f32, tag="ee")
                    rhs = bpool.tile([128, 4, 136], bf16, tag="rhs")
                    qkb = bpool.tile([128, 4, 128], f32, tag="qkb")
                    nc.vector.tensor_mul(qkb[:, 0:nb, :], qdg[:, t:t + nb, 0:128],
                                         kvb[:, t:t + nb, 0:128])
                    nc.vector.reduce_sum(
                        qe[:, 0:nb, 0:4],
                        qkb[:, 0:nb, :].rearrange("p t (h c) -> p t h c", h=4),
                        axis=AX.X)
                    nc.vector.tensor_mul(
                        qe[:, 0:nb, 4:8], qdg[:, t:t + nb, 128:132],
                        eas[:, tile0 + t:tile0 + t + nb].unsqueeze(2).broadcast_to([128, nb, 4]))
                    nc.scalar.activation(ee[:, 0:nb, :], qe[:, 0:nb, :], AF.Exp)
                    nc.vector.tensor_mul(rhs[:, 0:nb, 128:132], ee[:, 0:nb, 0:4],
                                         ee[:, 0:nb, 4:8])
                    nc.vector.tensor_mul(
                        rhs[:, 0:nb, 0:128].rearrange("p t (h c) -> p t h c", h=4),
                        kvb[:, t:t + nb, 128:256].rearrange("p t (h c) -> p t h c", h=4),
                        rhs[:, 0:nb, 128:132].unsqueeze(3).broadcast_to([128, nb, 4, 32]))
                    nc.vector.tensor_mul(
                        rhs[:, 0:nb, 132:136], rhs[:, 0:nb, 128:132],
                        eas[:, tile0 + t:tile0 + t + nb].unsqueeze(2).broadcast_to([128, nb, 4]))
                    for j in range(nb):
                        ti = plan.tiles[tile0 + t + j]
                        tg = tile0 + t + j
                        nc.tensor.matmul(aggs[ti["b"]][:], Ob[:, t + j, :], rhs[:, j, :],
                                         start=(tg == plan.first_tile[ti["b"]]),
                                         stop=(tg == plan.last_tile[ti["b"]]))
                    t += nb
                for b in blocks:
                    A = aggs[b]
                    den = spool.tile([128, 4], f32, tag="den")
                    nc.vector.tensor_scalar_add(den[:], A[:, 128:132], 1e-16)
                    rec = spool.tile([128, 4], f32, tag="rec")
                    nc.vector.reciprocal(rec[:], den[:])
                    nr = fpool.tile([128, 128], f32, tag="nr")
                    nc.vector.tensor_mul(
                        nr[:].rearrange("p (h c) -> p h c", h=4),
                        A[:, 0:128].rearrange("p (h c) -> p h c", h=4),
                        rec[:].unsqueeze(2).broadcast_to([128, 4, 32]))
                    exr = fpool.tile([128, 4], f16, tag="exr")
                    nc.vector.tensor_mul(exr[:], A[:, 132:136], rec[:])
                    tpe = ptp.tile([4, 128], f16, tag="tpe")
                    nc.tensor.transpose(tpe[:], exr[:], ident[:])
                    exrT = fpool.tile([4, 128], f16, tag="exrT")
                    nc.scalar.copy(exrT[:], tpe[:])
                    ps2 = pfin.tile([128, 128], f32, tag="pfin")
                    nc.tensor.matmul(ps2[:], h1Tls[:, b * 128:(b + 1) * 128], Ws1s[:],
                                     start=True, stop=False)
                    nc.tensor.matmul(ps2[:], ones1[:], brow1s[:], start=False, stop=False)
                    nc.tensor.matmul(ps2[:], exrT[:], We1bds[:], start=False, stop=True)
                    h2p = fpool.tile([128, 128], f32, tag="h2p")
                    nc.vector.tensor_add(h2p[:], ps2[:], nr[:])
                    h2s = fpool.tile([128, 128], f16, tag="h2s")
                    nc.vector.tensor_scalar_max(h2s[:], h2p[:], 0.0)
                    tp = ptp.tile([128, 128], f16, tag="tph")
                    nc.tensor.transpose(tp[:], h2s[:], ident[:])
                    nc.scalar.copy(h2T[:, b * 128:(b + 1) * 128], tp[:])

        # ---- phase 3: classifier on h2T
        with ExitStack() as c2:
            cpool = c2.enter_context(tc.tile_pool(name="cls", bufs=3))
            wpool = c2.enter_context(tc.tile_pool(name="clw", bufs=1))
            pc1 = c2.enter_context(tc.tile_pool(name="pc", bufs=2, space="PSUM"))
            Wc1s = wpool.tile([128, 128], f16); nc.sync.dma_start(Wc1s[:], Wc1[:, :])
            Wc2s = wpool.tile([128, 64], f16); nc.sync.dma_start(Wc2s[:], Wc2[:, :])
            Wc3s = wpool.tile([64, 128], f16); nc.sync.dma_start(Wc3s[:], Wc3[:, :])
            bc1s = wpool.tile([128, 1], f32); nc.sync.dma_start(bc1s[:], bc1[:, :])
            bc2s = wpool.tile([64, 1], f32); nc.sync.dma_start(bc2s[:], bc2[:, :])
            CB = 512
            for n0 in range(0, NLOC, CB):
                ps1 = pc1.tile([128, CB], f32, tag="c1")
                nc.tensor.matmul(ps1[:], Wc1s[:], h2T[:, n0:n0 + CB], start=True, stop=True)
                c1 = cpool.tile([128, CB], f16, tag="c1s")
                nc.scalar.activation(c1[:], ps1[:], AF.Relu, bias=bc1s[:])
                ps2 = pc1.tile([64, CB], f32, tag="c2")
                nc.tensor.matmul(ps2[:], Wc2s[:], c1[:], start=True, stop=True)
                c2s = cpool.tile([64, CB], f16, tag="c2s")
                nc.scalar.activation(c2s[:], ps2[:], AF.Relu, bias=bc2s[:])
                ps3 = pc1.tile([1, CB], f32, tag="c3")
                nc.tensor.matmul(ps3[:], Wc3s[:, 0:1], c2s[:], start=True, stop=True)
                nc.scalar.activation(outrow[:, n0:n0 + CB], ps3[:], AF.Copy,
                                     bias=float(bc3))
            nc.sync.dma_start(outv[:, :], outrow[:])
    nc.compile()
    return nc


# ------------------------------------------------------------------ host glue

def _inputs_l0(plan, inp):
    cfg = plan.cfg
    x = np.asarray(inp["x"], np.float32)
    ea = np.asarray(inp["edge_attr"], np.float32)[:, 0]
    NT = plan.NT
    # folded weights
    Mcat = np.zeros((HC, 12), np.float32)
    Wk0, We0 = np.asarray(inp["Wk0"], np.float32), np.asarray(inp["We0"], np.float32)[0]
    for h in range(H):
        for j, v in enumerate([Wk0[0], Wk0[1], We0]):
            Mcat[h * C:(h + 1) * C, h * 3 + j] = v[h * C:(h + 1) * C]
    A0 = (np.vstack([np.asarray(inp["Wq0"], np.float32),
                     np.asarray(inp["bq0"], np.float32)[None]]) @ Mcat) / SQC
    A0 = np.vstack([A0, np.zeros((1, 12), np.float32)]).astype(np.float16)
    V4 = np.zeros((16, HC), np.float32)
    Wv0 = np.asarray(inp["Wv0"], np.float32)
    for j, v in enumerate([Wv0[0], Wv0[1], We0, np.zeros(HC, np.float32)]):
        V4[j * 4:(j + 1) * 4, :] = head_block(v)
    V4 = V4.astype(np.float16)
    Wsk0 = np.vstack([np.asarray(inp["Ws0"], np.float32),
                      (np.asarray(inp["bs0"], np.float32) +
                       np.asarray(inp["bv0"], np.float32))[None],
                      np.zeros((1, HC), np.float32)]).astype(np.float16)
    I_np = np.zeros((256, 128), np.float16); I_np[:128] = np.eye(128)
    Ib_np = np.zeros((256, 128), ml_dtypes.bfloat16); Ib_np[:128] = np.eye(128)

    maps = []
    for ci, pcd in enumerate(plan.cores):
        cd = pcd["core"]
        bn = cd["blocknodes"].reshape(-1)  # packed-order global-local node ids
        xl = np.zeros((cfg.NLOC, 2), np.float32)
        valid = bn >= 0
        xl[valid] = x[bn[valid] + ci * cfg.NPC]
        xT1 = np.zeros((4, cfg.NLOC), np.float32)
        xT1[0] = xl[:, 0]; xT1[1] = xl[:, 1]; xT1[2] = 1.0
        u4 = np.zeros((NT * 128, 4), np.float32)
        pos, eid = pcd["positions"], pcd["e_ids"]
        gsrc = cd["s"][eid]
        u4[pos, 0] = x[gsrc, 0]; u4[pos, 1] = x[gsrc, 1]
        u4[pos, 2] = ea[cd["gmask"][eid]]; u4[:, 3] = 1.0
        maps.append({
            "xT1": xT1.astype(np.float16),
            "dstw": wrap16(pcd["dstloc"]),
            "qidxw": wrap16(pcd["qidx"]),
            "u4": u4.reshape(NT, 128, 4).transpose(1, 0, 2).astype(np.float16).copy(),
            "I_t": I_np, "Ib_t": Ib_np,
            "A0": A0, "V4": V4, "Wsk0": Wsk0,
        })
    return maps


def _inputs_l1(plan, inp, h1g16):
    cfg = plan.cfg
    NT = plan.NT
    ea = np.asarray(inp["edge_attr"], np.float32)[:, 0]
    Wq1 = np.asarray(inp["Wq1"], np.float32); bq1 = np.asarray(inp["bq1"], np.float32)
    We1 = np.asarray(inp["We1"], np.float32)[0]
    # qWe combination: qWe[n,h] = sum_c q1[n,hc]*We1[hc]  (q1 = h@Wq1+bq1)
    M = np.zeros((HC, 4), np.float32)
    for h in range(H):
        M[h * C:(h + 1) * C, h] = We1[h * C:(h + 1) * C]
    Wqc = np.concatenate([Wq1 / SQC, (Wq1 @ M) / SQC], axis=1)          # [128,132]
    qbias = np.concatenate([bq1 / SQC, (bq1 @ M) / SQC])[None, :]       # [1,132]
    Wkv = np.concatenate([np.asarray(inp["Wk1"], np.float32),
                          np.asarray(inp["Wv1"], np.float32)], axis=1)  # [128,256]
    brow1 = (np.asarray(inp["bs1"], np.float32) + np.asarray(inp["bv1"], np.float32))[None, :]
    We1bd = head_block(We1 / 1.0)  # [4, 128]
    I_np = np.zeros((256, 128), np.float16); I_np[:128] = np.eye(128)
    Ib_np = np.zeros((256, 128), ml_dtypes.bfloat16); Ib_np[:128] = np.eye(128)
    h1Tg = h1g16.T.copy()

    maps = []
    for ci, pcd in enumerate(plan.cores):
        cd = pcd["core"]
        bn = cd["blocknodes"].reshape(-1)
        h1l = np.zeros((cfg.NLOC, HC), np.float16)
        valid = bn >= 0
        h1l[valid] = h1g16[bn[valid] + ci * cfg.NPC]
        eav = np.zeros(NT * 128, np.float32)
        pos, eid = pcd["positions"], pcd["e_ids"]
        eav[pos] = ea[cd["gmask"][eid]]
        maps.append({
            "h1Tg": h1Tg, "h1Tl": h1l.T.copy(),
            "srcw": wrap16(pcd["srcidx"]), "dstw": wrap16(pcd["dstloc"]),
            "qidxw": wrap16(pcd["qidx"]),
            "ea16": eav.reshape(NT, 128).T.astype(np.float16).copy(),
            "I_t": I_np, "Ib_t": Ib_np,
            "Wkv": Wkv.astype(np.float16),
            "Wqc": Wqc.astype(np.float16),
            "qbias": qbias.astype(np.float16),
            "Ws1": np.asarray(inp["Ws1"], np.float32).astype(np.float16),
            "brow1": brow1.astype(np.float16),
            "We1bd": We1bd.astype(np.float16),
            "Wc1i": np.asarray(inp["Wc1"], np.float32).astype(np.float16),
            "Wc2i": np.asarray(inp["Wc2"], np.float32).astype(np.float16),
            "Wc3i": np.pad(np.asarray(inp["Wc3"], np.float32), ((0, 0), (0, 127))).astype(np.float16),
            "bc1i": np.asarray(inp["bc1"], np.float32)[:, None],
            "bc2i": np.asarray(inp["bc2"], np.float32)[:, None],
        })
    return maps


_CACHE = {}


def _run(cfg, inp, profile=False):
    src = np.asarray(inp["edge_index"][0], np.int64)
    dst = np.asarray(inp["edge_index"][1], np.int64)
    key = "plan"
    if key not in _CACHE:
        _CACHE[key] = make_plan(cfg, src, dst)
    plan = _CACHE[key]

    if "nc0" not in _CACHE:
        nc0 = bacc.Bacc("TRN2", target_bir_lowering=False, debug=False)
        _CACHE["nc0"] = build_l0(nc0, plan)
    maps0 = _inputs_l0(plan, inp)
    import time as _t
    _t0 = _t.time()
    res0 = run_bass_kernel_spmd(_CACHE["nc0"], maps0, core_ids=list(range(cfg.NCORES)))
    _tA = _t.time() - _t0
    global _LAST_WALL_A; _LAST_WALL_A = _tA
    if os.environ.get("KBENCH_PHASE") == "0":
        h1g16 = np.zeros((cfg.NGLOB, HC), np.float16)
        for ci, pcd in enumerate(plan.cores):
            bn = pcd["core"]["blocknodes"].reshape(-1)
            valid = bn >= 0
            h1g16[bn[valid] + ci * cfg.NPC] = np.asarray(res0.results[ci]["h1out"])[valid]
        np.save("/root/problem/h1_dev.npy", h1g16)
        return np.zeros((cfg.N, 1), np.float32)

    # host h1 exchange
    h1g16 = np.zeros((cfg.NGLOB, HC), np.float16)
    for ci, pcd in enumerate(plan.cores):
        bn = pcd["core"]["blocknodes"].reshape(-1)
        valid = bn >= 0
        h1g16[bn[valid] + ci * cfg.NPC] = np.asarray(res0.results[ci]["h1out"])[valid]

    if "nc1" not in _CACHE:
        nc1 = bacc.Bacc("TRN2", target_bir_lowering=False, debug=False)
        _CACHE["nc1"] = build_l1(nc1, plan, float(np.asarray(inp["bc3"], np.float32)[0]))
    maps1 = _inputs_l1(plan, inp, h1g16)
    _t0 = _t.time()
    res1 = run_bass_kernel_spmd(_CACHE["nc1"], maps1, core_ids=list(range(cfg.NCORES)))
    _tB = _t.time() - _t0
    global _LAST_WALL_B; _LAST_WALL_B = _tB

    out = np.zeros((cfg.N, 1), np.float32)
    for ci, pcd in enumerate(plan.cores):
        bn = pcd["core"]["blocknodes"].reshape(-1)
        valid = bn >= 0
        out[bn[valid] + ci * cfg.NPC, 0] = np.asarray(res1.results[ci]["outv"])[0][valid]
    return out


def kernel(**inputs) -> np.ndarray:
    return _run(FULL, inputs)

